# revision 27
# baseline (speedup 1.0000x reference)
"""Trainium2 Bass kernel for nn_DifferentiableAggregation (segment_reduce).

Computes, for batch of 8192 segments over 8388608 sub-images:
    s0[g]  = sum over i with idx_i == g of sub_logits[i, 0]
    s12[g] = sum over i with idx_i == g of (sub_logits[i, 1] + sub_logits[i, 2])
    out[g] = [log(sigmoid(10*(1-s12[g])) + 1e-10),
              log(sigmoid(10*(5-s0[g]))  + 1e-10)]

Strategy: shard the sub-image axis across 8 NeuronCores. Each core does a
local segment-sum via one-hot matmuls accumulating in PSUM (index split as
g = r*64 + q with r = idx>>6 on the 128 PSUM partitions and q = idx&63 in
the free dim), then an AllReduce of the [128, 128] partial and the
sigmoid/log epilogue on the scalar engine.
"""

import sys

sys.path.insert(0, "/opt/trn_rl_repo")

import numpy as np

from concourse import bass, bacc, mybir, tile
from concourse.bass_utils import run_bass_kernel_spmd

N_CORES = 8
TOTAL = 8388608
SHARD = TOTAL // N_CORES  # 1048576
BATCH = 8192
P = 128
F32 = mybir.dt.float32

K_SHARP = 10.0
EPS = 1e-10


def build_nc(to_count, ti):
    """Build + compile the SPMD bass program. Per core handles
    to_count * 128 * ti values."""
    shard = to_count * P * ti
    nc = bacc.Bacc(
        "TRN2",
        debug=False,
        target_bir_lowering=False,
        num_devices=N_CORES,
    )
    v_in = nc.dram_tensor("v", [shard * 3], F32, kind="ExternalInput")
    r_in = nc.dram_tensor("ridx", [shard], F32, kind="ExternalInput")
    q_in = nc.dram_tensor("qidx", [shard], F32, kind="ExternalInput")
    io128_in = nc.dram_tensor("iota128", [P, 128], F32, kind="ExternalInput")
    io64_in = nc.dram_tensor("iota64", [P, 64], F32, kind="ExternalInput")
    out_part = nc.dram_tensor("part", [P, 128], F32, kind="ExternalOutput")
    out_logits = nc.dram_tensor("logits", [2, BATCH], F32, kind="ExternalOutput")

    with tile.TileContext(nc) as tc:
        _kernel_body(
            tc, to_count, ti, v_in, r_in, q_in, io128_in, io64_in, out_part, out_logits
        )
    nc.compile()
    return nc


def _kernel_body(tc, to_count, ti, v_in, r_in, q_in, io128_in, io64_in,
                 out_part, out_logits):
    nc = tc.nc
    add = mybir.AluOpType.add
    is_equal = mybir.AluOpType.is_equal
    mult = mybir.AluOpType.mult
    AF = mybir.ActivationFunctionType

    v3 = v_in.ap().rearrange("(o p f) -> o p f", p=P, f=ti * 3)
    rv = r_in.ap().rearrange("(o p f) -> o p f", p=P, f=ti)
    qv = q_in.ap().rearrange("(o p f) -> o p f", p=P, f=ti)

    S = min(32, ti)  # micro-tiles per batched build block
    assert ti % S == 0
    nb = ti // S

    with (
        tc.tile_pool(name="const", bufs=1) as cpool,
        tc.tile_pool(name="data", bufs=2) as dpool,
        tc.tile_pool(name="onehot", bufs=2) as bpool,
        tc.tile_pool(name="mid", bufs=3) as mpool,
        tc.tile_pool(name="psum", bufs=1, space="PSUM") as ppool,
        tc.tile_pool(name="epi", bufs=1) as epool,
        tc.tile_pool(name="dram", bufs=1, space="DRAM") as drampool,
    ):
        io128 = cpool.tile([P, 128], F32)
        nc.sync.dma_start(io128[:], io128_in.ap())
        io64 = cpool.tile([P, 64], F32)
        nc.sync.dma_start(io64[:], io64_in.ap())
        io128b = io128[:].rearrange("p (o k) -> p o k", o=1).to_broadcast(
            [P, S, 128]
        )
        io64b = io64[:].rearrange("p (o k) -> p o k", o=1).to_broadcast([P, S, 64])

        acc = ppool.tile([P, 128], F32)

        for to in range(to_count):
            vt = dpool.tile([P, ti * 3], F32, tag="vt")
            nc.sync.dma_start(vt[:], v3[to])
            rt = dpool.tile([P, ti], F32, tag="rt")
            nc.sync.dma_start(rt[:], rv[to])
            qt = dpool.tile([P, ti], F32, tag="qt")
            nc.sync.dma_start(qt[:], qv[to])

            vt3 = vt[:].rearrange("p (t c) -> p t c", c=3)
            c12 = dpool.tile([P, ti], F32, tag="c12")
            nc.vector.tensor_tensor(c12[:], vt3[:, :, 1], vt3[:, :, 2], add)
            c0 = dpool.tile([P, ti], F32, tag="c0")
            nc.vector.tensor_copy(c0[:], vt3[:, :, 0])

            for b in range(nb):
                sl = slice(b * S, (b + 1) * S)
                # batched r one-hot: B3[p, j, k] = (r[p, j] == k)
                B_all = bpool.tile([P, S * 128], F32, tag="B")
                B3 = B_all[:].rearrange("p (j k) -> p j k", k=128)
                rb = (
                    rt[:, sl]
                    .rearrange("p (j o) -> p j o", o=1)
                    .to_broadcast([P, S, 128])
                )
                nc.vector.tensor_tensor(B3, rb, io128b, is_equal)

                # batched q difference on gpsimd (Pool has no compare ops)
                D_all = mpool.tile([P, S * 64], F32, tag="D")
                D3 = D_all[:].rearrange("p (j k) -> p j k", k=64)
                qb = (
                    qt[:, sl]
                    .rearrange("p (j o) -> p j o", o=1)
                    .to_broadcast([P, S, 64])
                )
                nc.gpsimd.tensor_tensor(D3, qb, io64b, mybir.AluOpType.subtract)

                # q one-hot on the (otherwise idle) scalar engine:
                # relu(1 - |D|) == (D == 0) for integer-valued D
                AB_all = mpool.tile([P, S * 64], F32, tag="AB")
                AB3 = AB_all[:].rearrange("p (j k) -> p j k", k=64)
                nc.scalar.activation(AB3, D3, AF.Abs, bias=0.0, scale=1.0)
                OHQ_all = mpool.tile([P, S * 64], F32, tag="OHQ")
                OHQ3 = OHQ_all[:].rearrange("p (j k) -> p j k", k=64)
                nc.scalar.activation(OHQ3, AB3, AF.Relu, bias=1.0, scale=-1.0)

                # batched VQ: onehot_q * value, channel-major layout so both
                # multiplies write contiguous panels (strided writes halve
                # DVE throughput); the matmul rhs picks the two panels with a
                # 2-group strided AP.
                VQ_all = bpool.tile([P, 2 * S * 64], F32, tag="VQ")
                VQ4 = VQ_all[:].rearrange("p (c j k) -> p c j k", c=2, k=64)
                c0b = (
                    c0[:, sl]
                    .rearrange("p (j o) -> p j o", o=1)
                    .to_broadcast([P, S, 64])
                )
                nc.vector.tensor_tensor(VQ4[:, 0], c0b, OHQ3, mult)
                c12b = (
                    c12[:, sl]
                    .rearrange("p (j o) -> p j o", o=1)
                    .to_broadcast([P, S, 64])
                )
                nc.vector.tensor_tensor(VQ4[:, 1], c12b, OHQ3, mult)

                VQr = VQ_all[:].rearrange("p (c j k) -> p j c k", c=2, k=64)
                for j in range(S):
                    first = to == 0 and b == 0 and j == 0
                    last = to == to_count - 1 and b == nb - 1 and j == S - 1
                    nc.tensor.matmul(
                        acc[:],
                        lhsT=B3[:, j, :],
                        rhs=VQr[:, j],
                        start=first,
                        stop=last,
                    )

        # Drain PSUM, publish this core's partial (debug / fallback)
        s_sb = epool.tile([P, 128], F32)
        nc.vector.tensor_copy(s_sb[:], acc[:])
        nc.sync.dma_start(out_part.ap(), s_sb[:])

        # AllReduce partials across the 8 cores (DRAM bounce buffers)
        din = drampool.tile([P, 128], F32)
        dout = drampool.tile([P, 128], F32)
        nc.gpsimd.dma_start(din[:], s_sb[:])
        nc.gpsimd.collective_compute(
            "AllReduce",
            add,
            replica_groups=[list(range(N_CORES))],
            ins=[din.opt()],
            outs=[dout.opt()],
        )
        sf = epool.tile([P, 128], F32)
        nc.gpsimd.dma_start(sf[:], dout[:])

        # Epilogue: out_c = log(sigmoid(z) + eps), z = -10*s + bias_c.
        # sigmoid computed exactly as 1/(1 + exp(-z)) (ACT exp table +
        # accurate DVE reciprocal); -z clamped at 88 to avoid exp
        # overflow (beyond that sigmoid+eps == eps in fp32 anyway).
        # exp and ln share one ACT table set, so no table swapping.
        beps = epool.tile([P, 1], F32)
        nc.vector.memset(beps[:], EPS)

        def logsig(out_ap, s_ap, zbias):
            mz = epool.tile([P, 64], F32, tag="mz")
            nc.vector.tensor_scalar(mz[:], s_ap, K_SHARP, -zbias,
                                    mybir.AluOpType.mult, mybir.AluOpType.add)
            nc.vector.tensor_scalar(mz[:], mz[:], 88.0, None,
                                    mybir.AluOpType.min)
            w = epool.tile([P, 64], F32, tag="w")
            nc.scalar.activation(w[:], mz[:], AF.Exp, bias=0.0, scale=1.0)
            nc.vector.tensor_scalar(w[:], w[:], 1.0, None,
                                    mybir.AluOpType.add)
            r = epool.tile([P, 64], F32, tag="r")
            nc.vector.reciprocal(r[:], w[:])
            nc.scalar.activation(out_ap, r[:], AF.Ln, bias=beps[:], scale=1.0)

        o1 = epool.tile([P, 64], F32)
        logsig(o1[:], sf[:, 64:128], K_SHARP)
        o0 = epool.tile([P, 64], F32)
        logsig(o0[:], sf[:, 0:64], 5.0 * K_SHARP)

        ol = out_logits.ap().rearrange("w (p t) -> w p t", p=P, t=BATCH // P)
        nc.sync.dma_start(ol[0], o1[:])
        nc.sync.dma_start(ol[1], o0[:])


_NC_CACHE = {}


def _get_nc(to_count, ti):
    key = (to_count, ti)
    if key not in _NC_CACHE:
        _NC_CACHE[key] = build_nc(to_count, ti)
    return _NC_CACHE[key]


def make_in_maps(sub_logits, original_indices, to_count, ti):
    shard = to_count * P * ti
    n = shard * N_CORES
    idx = np.asarray(original_indices).astype(np.int32)
    v = np.ascontiguousarray(np.asarray(sub_logits, dtype=np.float32)).reshape(-1)
    r_f = (idx >> 6).astype(np.float32)
    q_f = (idx & 63).astype(np.float32)
    io128 = np.ascontiguousarray(
        np.broadcast_to(np.arange(128, dtype=np.float32), (P, 128))
    )
    io64 = np.ascontiguousarray(
        np.broadcast_to(np.arange(64, dtype=np.float32), (P, 64))
    )
    vs = v.reshape(N_CORES, shard * 3)
    rs = r_f.reshape(N_CORES, shard)
    qs = q_f.reshape(N_CORES, shard)
    return [
        {
            "v": vs[c],
            "ridx": rs[c],
            "qidx": qs[c],
            "iota128": io128,
            "iota64": io64,
        }
        for c in range(N_CORES)
    ]


def kernel(sub_logits, original_indices, batch_size=None, _trace=False):
    to_count, ti = 16, 512
    nc = _get_nc(to_count, ti)
    in_maps = make_in_maps(sub_logits, original_indices, to_count, ti)
    res = run_bass_kernel_spmd(
        nc, in_maps, core_ids=list(range(N_CORES)), trace=_trace
    )
    logits = res.results[0]["logits"]
    out = np.stack([logits[0], logits[1]], axis=1).astype(np.float32)
    if _trace:
        kernel._last_results = res
    return out
